# revision 38
# baseline (speedup 1.0000x reference)
"""Trainium2 Bass kernel for nn_DistHead (block-diagonal molecule attention).

out = softmax_blockdiag(Q K^T / sqrt(H)) * exp(-invr0 * cdist(Z, Z)) @ V
with Q/K/V = X @ W{q,k,v}^T, block-diagonal over 128 molecules of 64 atoms.

Sharding: 16 whole molecules (1024 rows) per core across 8 cores --
perfectly parallel, zero cross-core communication.

v3 design (vs the fp32-dist baseline):
- Distance^2 via a single fp16 K=13 gram matmul with hi/lo split-precision
  coordinate rows (accurate to ~1e-5 in invr0^2-scaled units); invr0 folded
  into the operands so both Exp activations use compile-time scale=-1
  (Q is negated on the host so exp(-1*x) works for the scores too).
- Transposed scores ST = -K Q^T (+512 cross-molecule via two aug rows on
  the q/k tiles, rows 64:66) so PV consumes wei^T directly as the matmul
  rhs -- no PE transposes, no per-tile psum->sbuf copies.
- Activations grouped by function (Sqrt x2 then Exp x4) so the ACT engine
  loads each table set exactly once.
- Row sums via one all-ones [128,64] lhsT matmul per half (cross-molecule
  entries of exp are exactly 0); normalization + final transpose on host.
- DMA issues spread across sync/gpsimd so the scalar engine only runs
  activations and table loads.

Self-contained: hardcodes shapes from the problem spec; only imports
concourse from /opt/trn_rl_repo.
"""

import sys

if "/opt/trn_rl_repo" not in sys.path:
    sys.path.insert(0, "/opt/trn_rl_repo")

import numpy as np

N, E, H = 8192, 256, 64          # atoms, embedding, head size
NSEG, SEG = 128, 64              # molecules, atoms per molecule
NCORES = 8
RPC = N // NCORES                # rows per core (1024 = 16 molecules)
NT = RPC // 128                  # 128-row tiles per core (2 molecules each)
EC = E // 128                    # embedding chunks of 128
HF = NT // 2                     # tiles per half
KD = 13                          # distance gram contraction rows
EPS = 3e-5                       # sqrt bias: floor for gram rounding noise

_cache = {}


def _build_nc():
    import concourse.bacc as bacc
    import concourse.tile as tile
    from concourse import mybir

    f32 = mybir.dt.float32
    f16 = mybir.dt.float16
    AF = mybir.ActivationFunctionType

    nc = bacc.Bacc(None, target_bir_lowering=False, debug=False)

    # X chunks and all three weight matrices ride in one dram tensor; the
    # input stream is split across the sync and scalar hwdge queues (and the
    # consts over gpsimd\'s swdge queue) because DMA descriptor dispatch
    # (~25ns/descriptor, per queue) is the real input bottleneck.
    WB = EC * 192
    XW = WB + 2 * EC * 512
    xw_d = nc.dram_tensor("xw", [128, XW], f16, kind="ExternalInput")
    zab_d = nc.dram_tensor("zab", [2 * KD, RPC], f16, kind="ExternalInput")
    aug_d = nc.dram_tensor("aug", [2, 2 * RPC], f16, kind="ExternalInput")
    yt_d = nc.dram_tensor("yt", [H, RPC], f32, kind="ExternalOutput")
    rs_d = nc.dram_tensor("rs", [1, RPC], f32, kind="ExternalOutput")

    with tile.TileContext(nc) as tc:
        with (
            tc.tile_pool(name="consts", bufs=1) as consts,
            tc.tile_pool(name="sb", bufs=1) as sb,
            tc.tile_pool(name="ps", bufs=2, space="PSUM") as ps,
        ):
            # SBUF tiles
            xw = consts.tile([128, XW], f16, tag="xw")
            za = consts.tile([KD, RPC], f16, tag="za")
            zb = consts.tile([KD, RPC], f16, tag="zb")
            onesw = consts.tile([128, H], f16, tag="onesw")
            epsb = consts.tile([128, 1], f32, tag="epsb")
            qk = sb.tile([H + 2, 2 * RPC], f16, tag="qk")
            dist = sb.tile([128, NT, 128], f16, tag="dist")
            sadd = sb.tile([128, NT, 128], f16, tag="sadd")
            e16 = sb.tile([128, NT, 128], f16, tag="e16")
            w16 = sb.tile([128, NT, 128], f16, tag="w16")
            v16 = sb.tile([128, NT * H], f16, tag="v16")
            ysb = sb.tile([H, NT, 128], f32, tag="ysb")
            rssb = sb.tile([1, RPC], f32, tag="rssb")
            warm = sb.tile([128, 1], f32, tag="warm")
            qkv = qk.rearrange("p (s n) -> p s n", s=2)

            def xt(h, c):
                o = WB + (h * EC + c) * 512
                return xw[:, o : o + 512]

            def xtile(t, c):
                o = WB + (t // HF * EC + c) * 512 + (t % HF) * 128
                return xw[:, o : o + 128]

            def wall(c, lo, hi):
                o = c * 192
                return xw[:, o + lo : o + hi]

            # DMA issues across three independent descriptor dispatchers.
            nc.sync.dma_start(out=xw[:, 0 : WB + 1024], in_=xw_d[:, 0 : WB + 1024])
            nc.scalar.dma_start(out=xw[:, WB + 1024 : XW], in_=xw_d[:, WB + 1024 : XW])
            nc.gpsimd.memset(onesw, 1.0)
            nc.gpsimd.memset(epsb, EPS)
            nc.gpsimd.dma_start(out=za, in_=zab_d[0:KD, :])
            nc.gpsimd.dma_start(out=zb, in_=zab_d[KD : 2 * KD, :])
            nc.gpsimd.dma_start(out=qk[H : H + 2, :], in_=aug_d[:, :])

            # Warm the Sqrt table so its load runs during the DMA prologue.
            # Activations are ordered sqrt* then exp* (every Sqrt<->Exp
            # transition costs a 1.3us table load).
            nc.scalar.activation(out=warm, in_=epsb, func=AF.Sqrt)

            # --- PE stream -------------------------------------------------
            # Distance gram first (za/zb land before X), then projections.
            d_ps = ps.tile([128, NT, 128], f32, tag="big")
            for t in range(NT):
                rt = slice(t * 128, (t + 1) * 128)
                nc.tensor.matmul(d_ps[:, t, :], lhsT=za[:, rt], rhs=zb[:, rt], start=True, stop=True)

            # K and Q projections write one [64, 1024] psum tile per half
            # (K cols 0:512, Q cols 512:1024) so a single strided DVE cast
            # per half lands both into the combined qk tile.
            kq_ps = {}
            for h in range(EC):
                p = ps.tile([H, 1024], f32, tag="kq", name=f"kq{h}")
                for iw in (1, 0):
                    for c in range(EC):
                        nc.tensor.matmul(
                            p[:, (1 - iw) * 512 : (2 - iw) * 512],
                            lhsT=wall(c, iw * H, (iw + 1) * H), rhs=xt(h, c),
                            start=(c == 0), stop=(c == EC - 1),
                        )
                kq_ps[h] = p

            for h in range(EC):
                nc.vector.tensor_copy(
                    out=qkv[0:H, :, h * 512 : (h + 1) * 512],
                    in_=kq_ps[h].rearrange("p (s n) -> p s n", s=2),
                )

            # Transposed scores: ST = -K Q^T + {0 | +512} via aug rows.
            st_ps = ps.tile([128, NT, 128], f32, tag="big")
            for t in range(HF):
                rt = slice(t * 128, (t + 1) * 128)
                nc.tensor.matmul(st_ps[:, t, :], lhsT=qk[:, rt], rhs=qk[:, RPC + rt.start : RPC + rt.stop], start=True, stop=True)

            # V projection, row-major [atom, H] for the PV lhsT.
            v_ps = ps.tile([128, NT * H], f32, tag="kq")
            for t in range(NT):
                for c in range(EC):
                    nc.tensor.matmul(
                        v_ps[:, t * H : (t + 1) * H],
                        lhsT=xtile(t, c), rhs=wall(c, 2 * H, 3 * H),
                        start=(c == 0), stop=(c == EC - 1),
                    )
            nc.vector.tensor_copy(out=v16, in_=v_ps)

            for t in range(HF, NT):
                rt = slice(t * 128, (t + 1) * 128)
                nc.tensor.matmul(st_ps[:, t, :], lhsT=qk[:, rt], rhs=qk[:, RPC + rt.start : RPC + rt.stop], start=True, stop=True)

            # --- ACT: sqrt x2 then exp x4 (one table load each) ------------
            for hh in range(2):
                hs = slice(hh * HF, (hh + 1) * HF)
                nc.scalar.activation(out=dist[:, hs, :], in_=d_ps[:, hs, :], func=AF.Sqrt, bias=epsb)

            # sadd = ST + dist (DVE): folds the decay into the score exp.
            for hh in range(2):
                hs = slice(hh * HF, (hh + 1) * HF)
                nc.vector.tensor_add(out=sadd[:, hs, :], in0=st_ps[:, hs, :], in1=dist[:, hs, :])

            # e16 first (it unblocks the rowsum/rs chain), then the weights.
            for hh in range(2):
                hs = slice(hh * HF, (hh + 1) * HF)
                nc.scalar.activation(out=e16[:, hs, :], in_=st_ps[:, hs, :], func=AF.Exp, scale=-1.0)
            for hh in range(2):
                hs = slice(hh * HF, (hh + 1) * HF)
                nc.scalar.activation(out=w16[:, hs, :], in_=sadd[:, hs, :], func=AF.Exp, scale=-1.0)

            # Row sums, then PV.
            ro = []
            for hh in range(2):
                r_ps = ps.tile([H, 512], f32, tag="kq", name=f"r_ps{hh}")
                nc.tensor.matmul(
                    r_ps, lhsT=onesw, rhs=e16[:, hh * HF : (hh + 1) * HF, :],
                    start=True, stop=True,
                )
                ro.append(r_ps)
                nc.vector.tensor_copy(out=rssb[:, hh * 512 : (hh + 1) * 512], in_=r_ps[0:1, :])
            oT_ps = ps.tile([H, NT, 128], f32, tag="big")
            for t in range(NT):
                nc.tensor.matmul(
                    oT_ps[:, t, :], lhsT=v16[:, t * H : (t + 1) * H],
                    rhs=w16[:, t, :], start=True, stop=True,
                )

            # psum -> sbuf -> DRAM (normalization happens on the host);
            # yt halves dispatch from two different queues in parallel.
            nc.sync.dma_start(out=rs_d[:, :], in_=rssb)
            for hh in range(2):
                hs = slice(hh * HF, (hh + 1) * HF)
                nc.scalar.activation(out=ysb[:, hs, :], in_=oT_ps[:, hs, :], func=AF.Copy)
            nc.sync.dma_start(out=yt_d[:, 0:512], in_=ysb[:, 0:HF, :])
            nc.gpsimd.dma_start(out=yt_d[:, 512:1024], in_=ysb[:, HF:NT, :])

    nc.compile()
    return nc


def _get_nc():
    if "nc" not in _cache:
        _cache["nc"] = _build_nc()
    return _cache["nc"]


def _hilo(v):
    h = v.astype(np.float16).astype(np.float32)
    return h, v - h


def _prepare_in_maps(X, Z, Wk, Wq, Wv, invr0):
    X = np.ascontiguousarray(X, dtype=np.float32)
    Z = np.asarray(Z, dtype=np.float32)
    inv = np.float32(np.asarray(invr0).reshape(-1)[0])

    # [128, EC, N] fp16: partition p, chunk c -> X^T row c*128+p. Per core
    # this becomes [128, half, EC, 512], then the fused weight block
    # [-Wq^T*scale | Wk^T | Wv^T] ([128, EC, 192]) is appended along the
    # free axis so X + W ship as one 128-descriptor DMA.
    xt_full = X.T.reshape(EC, 128, N).transpose(1, 0, 2).astype(np.float16)
    scale = np.float32(H) ** np.float32(-0.5)
    wall = np.concatenate([-(Wq.T * scale), Wk.T, Wv.T], axis=1).astype(np.float32)
    w_flat = wall.reshape(EC, 128, 192).transpose(1, 0, 2).astype(np.float16).reshape(128, EC * 192)

    # Distance gram rows, invr0-scaled, hi/lo split so the fp16 matmul keeps
    # d2 accurate to ~EPS in scaled units.
    zs = Z * inv
    z2s = np.sum(Z * Z, axis=-1) * (inv * inv)
    z2h, z2l = _hilo(z2s)
    zh, zl = _hilo(zs)
    ones = np.ones(N, dtype=np.float32)
    a_rows = [z2h, z2l, ones, ones]
    b_rows = [ones, ones, z2h, z2l]
    for c in range(3):
        a_rows += [-2.0 * zh[:, c], -2.0 * zh[:, c], -2.0 * zl[:, c]]
        b_rows += [zh[:, c], zl[:, c], zh[:, c]]
    zab_full = np.ascontiguousarray(np.stack(a_rows + b_rows).astype(np.float16))

    # Aug rows: ST += 256 - 256*sig_k*sig_i = 0 same-molecule, +512 cross
    # -> exp(-ST) underflows to exactly 0 in fp16. Layout matches the
    # combined qk tile: [2, 2*RPC] with the k-side (a) rows in cols 0:RPC
    # and the q-side (b) rows in cols RPC:2*RPC.
    sig = np.where((np.arange(N) % 128) < SEG, 16.0, -16.0).astype(np.float32)
    onesN = np.ones(N, dtype=np.float32)
    aug_a = np.stack([256.0 * onesN, sig])
    aug_b = np.stack([onesN, -sig])

    in_maps = []
    for d in range(NCORES):
        s, e = d * RPC, (d + 1) * RPC
        xcore = (
            xt_full[:, :, s:e]
            .reshape(128, EC, 2, 512)
            .transpose(0, 2, 1, 3)
            .reshape(128, 2 * EC * 512)
        )
        in_maps.append(
            {
                "xw": np.ascontiguousarray(np.concatenate([w_flat, xcore], axis=1)),
                "zab": np.ascontiguousarray(zab_full[:, s:e]),
                "aug": np.ascontiguousarray(
                    np.concatenate([aug_a[:, s:e], aug_b[:, s:e]], axis=1).astype(np.float16)
                ),
            }
        )
    return in_maps


def _run(in_maps, trace=False, **kwargs):
    from concourse.bass_utils import run_bass_kernel_spmd

    nc = _get_nc()
    return run_bass_kernel_spmd(nc, in_maps, list(range(NCORES)), trace=trace, **kwargs)


def _numpy_fallback(X, Z, Wk, Wq, Wv, invr0, ptr):
    """Reference-exact fallback for ptr layouts other than 128 x 64."""
    X = np.asarray(X, dtype=np.float32)
    Z = np.asarray(Z, dtype=np.float32)
    n = X.shape[0]
    K = X @ Wk.T
    Q = X @ Wq.T
    V = X @ Wv.T
    seg = np.searchsorted(np.asarray(ptr)[1:], np.arange(n), side="right")
    out = np.zeros((n, Wk.shape[0]), dtype=np.float32)
    inv = float(np.asarray(invr0).reshape(-1)[0])
    hs = Wk.shape[0] ** -0.5
    for s in np.unique(seg):
        idx = np.nonzero(seg == s)[0]
        q, k, v, z = Q[idx], K[idx], V[idx], Z[idx]
        wei = (q @ k.T) * hs
        wei = wei - wei.max(axis=-1, keepdims=True)
        wei = np.exp(wei)
        wei /= wei.sum(axis=-1, keepdims=True)
        d2 = np.maximum(
            (z * z).sum(-1)[:, None] + (z * z).sum(-1)[None, :] - 2.0 * (z @ z.T), 0.0
        )
        dist = np.sqrt(np.where(d2 > 0, d2, 1.0)) * (d2 > 0)
        wei = wei * np.exp(-inv * dist)
        out[idx] = wei @ v
    return out


def kernel(X, Z, Wk, Wq, Wv, invr0, ptr):
    ptr = np.asarray(ptr)
    if not (
        X.shape == (N, E)
        and Wk.shape == (H, E)
        and ptr.shape == (NSEG + 1,)
        and np.array_equal(ptr, np.arange(NSEG + 1, dtype=ptr.dtype) * SEG)
    ):
        return _numpy_fallback(X, Z, Wk, Wq, Wv, invr0, ptr)

    in_maps = _prepare_in_maps(X, Z, Wk, Wq, Wv, invr0)
    res = _run(in_maps, trace=False)
    out = np.empty((N, H), dtype=np.float32)
    for d in range(NCORES):
        yt = res.results[d]["yt"]            # [H, RPC] unnormalized out^T
        rs = res.results[d]["rs"][0]         # [RPC] softmax denominators
        out[d * RPC : (d + 1) * RPC] = (yt / rs[None, :]).T
    return out


# revision 40
# speedup vs baseline: 1.0173x; 1.0173x over previous
"""Trainium2 Bass kernel for nn_DistHead (block-diagonal molecule attention).

out = softmax_blockdiag(Q K^T / sqrt(H)) * exp(-invr0 * cdist(Z, Z)) @ V
with Q/K/V = X @ W{q,k,v}^T, block-diagonal over 128 molecules of 64 atoms.

Sharding: 16 whole molecules (1024 rows) per core across 8 cores --
perfectly parallel, zero cross-core communication.

v3 design (vs the fp32-dist baseline):
- Distance^2 via a single fp16 K=13 gram matmul with hi/lo split-precision
  coordinate rows (accurate to ~1e-5 in invr0^2-scaled units); invr0 folded
  into the operands so both Exp activations use compile-time scale=-1
  (Q is negated on the host so exp(-1*x) works for the scores too).
- Transposed scores ST = -K Q^T (+512 cross-molecule via two aug rows on
  the q/k tiles, rows 64:66) so PV consumes wei^T directly as the matmul
  rhs -- no PE transposes, no per-tile psum->sbuf copies.
- Activations grouped by function (Sqrt x2 then Exp x4) so the ACT engine
  loads each table set exactly once.
- Row sums via one all-ones [128,64] lhsT matmul per half (cross-molecule
  entries of exp are exactly 0); normalization + final transpose on host.
- DMA issues spread across sync/gpsimd so the scalar engine only runs
  activations and table loads.

Self-contained: hardcodes shapes from the problem spec; only imports
concourse from /opt/trn_rl_repo.
"""

import sys

if "/opt/trn_rl_repo" not in sys.path:
    sys.path.insert(0, "/opt/trn_rl_repo")

import numpy as np

N, E, H = 8192, 256, 64          # atoms, embedding, head size
NSEG, SEG = 128, 64              # molecules, atoms per molecule
NCORES = 8
RPC = N // NCORES                # rows per core (1024 = 16 molecules)
NT = RPC // 128                  # 128-row tiles per core (2 molecules each)
EC = E // 128                    # embedding chunks of 128
HF = NT // 2                     # tiles per half
KD = 13                          # distance gram contraction rows
EPS = 3e-5                       # sqrt bias: floor for gram rounding noise

_cache = {}


def _build_nc():
    import concourse.bacc as bacc
    import concourse.tile as tile
    from concourse import mybir

    f32 = mybir.dt.float32
    f16 = mybir.dt.float16
    AF = mybir.ActivationFunctionType

    nc = bacc.Bacc(None, target_bir_lowering=False, debug=False)

    # X chunks and all three weight matrices ride in one dram tensor; the
    # input stream is split across the sync and scalar hwdge queues (and the
    # consts over gpsimd\'s swdge queue) because DMA descriptor dispatch
    # (~25ns/descriptor, per queue) is the real input bottleneck.
    WB = EC * 192
    XW = WB + 2 * EC * 512
    xw_d = nc.dram_tensor("xw", [128, XW], f16, kind="ExternalInput")
    zab_d = nc.dram_tensor("zab", [2 * KD, RPC], f16, kind="ExternalInput")
    aug_d = nc.dram_tensor("aug", [2, 2 * RPC], f16, kind="ExternalInput")
    yt_d = nc.dram_tensor("yt", [H, RPC], f32, kind="ExternalOutput")
    rs_d = nc.dram_tensor("rs", [1, RPC], f32, kind="ExternalOutput")

    with tile.TileContext(nc) as tc:
        with (
            tc.tile_pool(name="consts", bufs=1) as consts,
            tc.tile_pool(name="sb", bufs=1) as sb,
            tc.tile_pool(name="ps", bufs=2, space="PSUM") as ps,
        ):
            # SBUF tiles
            xw = consts.tile([128, XW], f16, tag="xw")
            za = consts.tile([KD, RPC], f16, tag="za")
            zb = consts.tile([KD, RPC], f16, tag="zb")
            onesw = consts.tile([128, H], f16, tag="onesw")
            epsb = consts.tile([128, 1], f32, tag="epsb")
            qk = sb.tile([H + 2, 2 * RPC], f16, tag="qk")
            dist = sb.tile([128, NT, 128], f16, tag="dist")
            sadd = sb.tile([128, NT, 128], f16, tag="sadd")
            e16 = sb.tile([128, NT, 128], f16, tag="e16")
            w16 = sb.tile([128, NT, 128], f16, tag="w16")
            v16 = sb.tile([128, NT * H], f16, tag="v16")
            ysb = sb.tile([H, NT, 128], f32, tag="ysb")
            rssb = sb.tile([1, RPC], f32, tag="rssb")
            warm = sb.tile([128, 1], f32, tag="warm")
            qkv = qk.rearrange("p (s n) -> p s n", s=2)

            def xt(h, c):
                o = WB + (h * EC + c) * 512
                return xw[:, o : o + 512]

            def xtile(t, c):
                o = WB + (t // HF * EC + c) * 512 + (t % HF) * 128
                return xw[:, o : o + 128]

            def wall(c, lo, hi):
                o = c * 192
                return xw[:, o + lo : o + hi]

            # DMA issues across three independent descriptor dispatchers.
            nc.sync.dma_start(out=xw[:, 0 : WB + 1024], in_=xw_d[:, 0 : WB + 1024])
            nc.scalar.dma_start(out=xw[:, WB + 1024 : XW], in_=xw_d[:, WB + 1024 : XW])
            nc.gpsimd.memset(onesw, 1.0)
            nc.gpsimd.memset(epsb, EPS)
            nc.gpsimd.dma_start(out=za, in_=zab_d[0:KD, :])
            nc.gpsimd.dma_start(out=zb, in_=zab_d[KD : 2 * KD, :])
            nc.gpsimd.dma_start(out=qk[H : H + 2, :], in_=aug_d[:, :])

            # Warm the Sqrt table so its load runs during the DMA prologue.
            # Activations are ordered sqrt* then exp* (every Sqrt<->Exp
            # transition costs a 1.3us table load).
            nc.scalar.activation(out=warm, in_=epsb, func=AF.Sqrt)

            # --- PE stream -------------------------------------------------
            # Distance gram first (za/zb land before X), then projections.
            # high_priority pins the gram + sqrt chain at the front of the
            # PE/ACT queues -- the list scheduler otherwise risks parking it
            # behind the X-gated projections.
            d_ps = ps.tile([128, NT, 128], f32, tag="big")
            with tc.high_priority():
                for t in range(NT):
                    rt = slice(t * 128, (t + 1) * 128)
                    nc.tensor.matmul(d_ps[:, t, :], lhsT=za[:, rt], rhs=zb[:, rt], start=True, stop=True)
                for hh in range(2):
                    hs = slice(hh * HF, (hh + 1) * HF)
                    nc.scalar.activation(out=dist[:, hs, :], in_=d_ps[:, hs, :], func=AF.Sqrt, bias=epsb)

            # K and Q projections write one [64, 1024] psum tile per half
            # (K cols 0:512, Q cols 512:1024) so a single strided DVE cast
            # per half lands both into the combined qk tile.
            kq_ps = {}
            for h in range(EC):
                p = ps.tile([H, 1024], f32, tag="kq", name=f"kq{h}")
                for iw in (1, 0):
                    for c in range(EC):
                        nc.tensor.matmul(
                            p[:, (1 - iw) * 512 : (2 - iw) * 512],
                            lhsT=wall(c, iw * H, (iw + 1) * H), rhs=xt(h, c),
                            start=(c == 0), stop=(c == EC - 1),
                        )
                kq_ps[h] = p

            for h in range(EC):
                nc.vector.tensor_copy(
                    out=qkv[0:H, :, h * 512 : (h + 1) * 512],
                    in_=kq_ps[h].rearrange("p (s n) -> p s n", s=2),
                )

            # Transposed scores: ST = -K Q^T + {0 | +512} via aug rows.
            st_ps = ps.tile([128, NT, 128], f32, tag="big")
            for t in range(HF):
                rt = slice(t * 128, (t + 1) * 128)
                nc.tensor.matmul(st_ps[:, t, :], lhsT=qk[:, rt], rhs=qk[:, RPC + rt.start : RPC + rt.stop], start=True, stop=True)

            # V projection, row-major [atom, H] for the PV lhsT.
            v_ps = ps.tile([128, NT * H], f32, tag="kq")
            for t in range(NT):
                for c in range(EC):
                    nc.tensor.matmul(
                        v_ps[:, t * H : (t + 1) * H],
                        lhsT=xtile(t, c), rhs=wall(c, 2 * H, 3 * H),
                        start=(c == 0), stop=(c == EC - 1),
                    )
            nc.vector.tensor_copy(out=v16, in_=v_ps)

            for t in range(HF, NT):
                rt = slice(t * 128, (t + 1) * 128)
                nc.tensor.matmul(st_ps[:, t, :], lhsT=qk[:, rt], rhs=qk[:, RPC + rt.start : RPC + rt.stop], start=True, stop=True)

            # sadd = ST + dist (DVE): folds the decay into the score exp.
            for hh in range(2):
                hs = slice(hh * HF, (hh + 1) * HF)
                nc.vector.tensor_add(out=sadd[:, hs, :], in0=st_ps[:, hs, :], in1=dist[:, hs, :])

            # e16 first (it unblocks the rowsum/rs chain), then the weights.
            for hh in range(2):
                hs = slice(hh * HF, (hh + 1) * HF)
                nc.scalar.activation(out=e16[:, hs, :], in_=st_ps[:, hs, :], func=AF.Exp, scale=-1.0)
            for hh in range(2):
                hs = slice(hh * HF, (hh + 1) * HF)
                nc.scalar.activation(out=w16[:, hs, :], in_=sadd[:, hs, :], func=AF.Exp, scale=-1.0)

            # Row sums, then PV.
            ro = []
            for hh in range(2):
                r_ps = ps.tile([H, 512], f32, tag="kq", name=f"r_ps{hh}")
                nc.tensor.matmul(
                    r_ps, lhsT=onesw, rhs=e16[:, hh * HF : (hh + 1) * HF, :],
                    start=True, stop=True,
                )
                ro.append(r_ps)
                nc.vector.tensor_copy(out=rssb[:, hh * 512 : (hh + 1) * 512], in_=r_ps[0:1, :])
            oT_ps = ps.tile([H, NT, 128], f32, tag="big")
            for t in range(NT):
                nc.tensor.matmul(
                    oT_ps[:, t, :], lhsT=v16[:, t * H : (t + 1) * H],
                    rhs=w16[:, t, :], start=True, stop=True,
                )

            # psum -> sbuf -> DRAM (normalization happens on the host);
            # yt halves dispatch from two different queues in parallel.
            nc.sync.dma_start(out=rs_d[:, :], in_=rssb)
            for hh in range(2):
                hs = slice(hh * HF, (hh + 1) * HF)
                nc.scalar.activation(out=ysb[:, hs, :], in_=oT_ps[:, hs, :], func=AF.Copy)
            nc.sync.dma_start(out=yt_d[:, 0:512], in_=ysb[:, 0:HF, :])
            nc.gpsimd.dma_start(out=yt_d[:, 512:1024], in_=ysb[:, HF:NT, :])

    nc.compile()
    return nc


def _get_nc():
    if "nc" not in _cache:
        _cache["nc"] = _build_nc()
    return _cache["nc"]


def _hilo(v):
    h = v.astype(np.float16).astype(np.float32)
    return h, v - h


def _prepare_in_maps(X, Z, Wk, Wq, Wv, invr0):
    X = np.ascontiguousarray(X, dtype=np.float32)
    Z = np.asarray(Z, dtype=np.float32)
    inv = np.float32(np.asarray(invr0).reshape(-1)[0])

    # [128, EC, N] fp16: partition p, chunk c -> X^T row c*128+p. Per core
    # this becomes [128, half, EC, 512], then the fused weight block
    # [-Wq^T*scale | Wk^T | Wv^T] ([128, EC, 192]) is appended along the
    # free axis so X + W ship as one 128-descriptor DMA.
    xt_full = X.T.reshape(EC, 128, N).transpose(1, 0, 2).astype(np.float16)
    scale = np.float32(H) ** np.float32(-0.5)
    wall = np.concatenate([-(Wq.T * scale), Wk.T, Wv.T], axis=1).astype(np.float32)
    w_flat = wall.reshape(EC, 128, 192).transpose(1, 0, 2).astype(np.float16).reshape(128, EC * 192)

    # Distance gram rows, invr0-scaled, hi/lo split so the fp16 matmul keeps
    # d2 accurate to ~EPS in scaled units.
    zs = Z * inv
    z2s = np.sum(Z * Z, axis=-1) * (inv * inv)
    z2h, z2l = _hilo(z2s)
    zh, zl = _hilo(zs)
    ones = np.ones(N, dtype=np.float32)
    a_rows = [z2h, z2l, ones, ones]
    b_rows = [ones, ones, z2h, z2l]
    for c in range(3):
        a_rows += [-2.0 * zh[:, c], -2.0 * zh[:, c], -2.0 * zl[:, c]]
        b_rows += [zh[:, c], zl[:, c], zh[:, c]]
    zab_full = np.ascontiguousarray(np.stack(a_rows + b_rows).astype(np.float16))

    # Aug rows: ST += 256 - 256*sig_k*sig_i = 0 same-molecule, +512 cross
    # -> exp(-ST) underflows to exactly 0 in fp16. Layout matches the
    # combined qk tile: [2, 2*RPC] with the k-side (a) rows in cols 0:RPC
    # and the q-side (b) rows in cols RPC:2*RPC.
    sig = np.where((np.arange(N) % 128) < SEG, 16.0, -16.0).astype(np.float32)
    onesN = np.ones(N, dtype=np.float32)
    aug_a = np.stack([256.0 * onesN, sig])
    aug_b = np.stack([onesN, -sig])

    in_maps = []
    for d in range(NCORES):
        s, e = d * RPC, (d + 1) * RPC
        xcore = (
            xt_full[:, :, s:e]
            .reshape(128, EC, 2, 512)
            .transpose(0, 2, 1, 3)
            .reshape(128, 2 * EC * 512)
        )
        in_maps.append(
            {
                "xw": np.ascontiguousarray(np.concatenate([w_flat, xcore], axis=1)),
                "zab": np.ascontiguousarray(zab_full[:, s:e]),
                "aug": np.ascontiguousarray(
                    np.concatenate([aug_a[:, s:e], aug_b[:, s:e]], axis=1).astype(np.float16)
                ),
            }
        )
    return in_maps


def _run(in_maps, trace=False, **kwargs):
    from concourse.bass_utils import run_bass_kernel_spmd

    nc = _get_nc()
    return run_bass_kernel_spmd(nc, in_maps, list(range(NCORES)), trace=trace, **kwargs)


def _numpy_fallback(X, Z, Wk, Wq, Wv, invr0, ptr):
    """Reference-exact fallback for ptr layouts other than 128 x 64."""
    X = np.asarray(X, dtype=np.float32)
    Z = np.asarray(Z, dtype=np.float32)
    n = X.shape[0]
    K = X @ Wk.T
    Q = X @ Wq.T
    V = X @ Wv.T
    seg = np.searchsorted(np.asarray(ptr)[1:], np.arange(n), side="right")
    out = np.zeros((n, Wk.shape[0]), dtype=np.float32)
    inv = float(np.asarray(invr0).reshape(-1)[0])
    hs = Wk.shape[0] ** -0.5
    for s in np.unique(seg):
        idx = np.nonzero(seg == s)[0]
        q, k, v, z = Q[idx], K[idx], V[idx], Z[idx]
        wei = (q @ k.T) * hs
        wei = wei - wei.max(axis=-1, keepdims=True)
        wei = np.exp(wei)
        wei /= wei.sum(axis=-1, keepdims=True)
        d2 = np.maximum(
            (z * z).sum(-1)[:, None] + (z * z).sum(-1)[None, :] - 2.0 * (z @ z.T), 0.0
        )
        dist = np.sqrt(np.where(d2 > 0, d2, 1.0)) * (d2 > 0)
        wei = wei * np.exp(-inv * dist)
        out[idx] = wei @ v
    return out


def kernel(X, Z, Wk, Wq, Wv, invr0, ptr):
    ptr = np.asarray(ptr)
    if not (
        X.shape == (N, E)
        and Wk.shape == (H, E)
        and ptr.shape == (NSEG + 1,)
        and np.array_equal(ptr, np.arange(NSEG + 1, dtype=ptr.dtype) * SEG)
    ):
        return _numpy_fallback(X, Z, Wk, Wq, Wv, invr0, ptr)

    in_maps = _prepare_in_maps(X, Z, Wk, Wq, Wv, invr0)
    res = _run(in_maps, trace=False)
    out = np.empty((N, H), dtype=np.float32)
    for d in range(NCORES):
        yt = res.results[d]["yt"]            # [H, RPC] unnormalized out^T
        rs = res.results[d]["rs"][0]         # [RPC] softmax denominators
        out[d * RPC : (d + 1) * RPC] = (yt / rs[None, :]).T
    return out
